# revision 5
# baseline (speedup 1.0000x reference)
"""Trainium2 Bass kernel for multi-filter grayscale erosion (min-plus correlation).

out[b, y, x, f] = min_{dy, dx, c} ( x[b, y+dy, x+dx, c] - k[dy, dx, c, f] )

x: [32, 256, 256, 4] f32, k: [5, 5, 4, 8] f32 -> out: [32, 252, 252, 8] f32.

Strategy (per NeuronCore, data-parallel over batch, 4 images/core):
- De-interleave x into per-channel fp16 "planes" in SBUF laid out
  [p=126 partitions, j=2, x=256, b=4] with y = p + 126*j, one plane per
  (dy-shift, channel); the dy partition shift is folded into the HBM load.
- Per output filter f, the 100-tap min-plus reduction runs on THREE
  engines via three tap paths:
    A: ScalarE sub (activation bias add) -> VectorE min (TT, 2x fp16)
    B: VectorE sub (tensor_scalar, 4x fp16) -> VectorE min
    P: GPSIMD chain using min(acc,x-k) = acc - relu(acc-x+k):
       Pool TT-sub d=acc-x, ScalarE relu(d+k), Pool TT-sub acc-=r
       (Pool can't do TT-min on TRN2, but TT-sub is legal.)
- Independent accumulator chains (accD + nCh Pool chains) merge at
  filter end; the last merge writes straight into the [p,j,b,x,f]
  staging tile; DMA writes contiguous rows to HBM (fp16, upcast on host).
"""

import heapq
import numpy as np

_B, _H, _W, _C = 32, 256, 256, 4
_KH, _KW, _F = 5, 5, 8
_HO, _WO = _H - _KH + 1, _W - _KW + 1  # 252, 252
_NCORES = 8
_BL = _B // _NCORES  # 4
_P, _J = 126, 2  # y = p + 126*j

_MAX_WAITS = 1  # this walrus build rejects >1 sync wait per instruction


def _install_tile_drain_patch():
    import concourse.tile as _tile
    import concourse.mybir as mybir
    from concourse.vector_clock import ScopedClock

    if getattr(_tile.TileContext, "_drain_patch_installed", False):
        return

    def _patched_drain_and_barrier(self, tick_clock, wait_clock):
        nc = self.nc
        drain_inst = nc.sync.drain()
        wait_clock.add_sem_waits(
            drain_inst.ins, ScopedClock({None: tick_clock.global_clock})
        )
        si = drain_inst.ins.sync_info
        waits = list(si.on_wait) if si and si.on_wait else []
        if len(waits) > _MAX_WAITS:
            drain_inst.ins.sync_info = mybir.SyncInfo(
                on_wait=waits[:_MAX_WAITS], on_update=list(si.on_update or [])
            )
            for i in range(_MAX_WAITS, len(waits), _MAX_WAITS):
                d = nc.sync.drain()
                d.ins.sync_info = mybir.SyncInfo(
                    on_wait=waits[i : i + _MAX_WAITS], on_update=[]
                )
        nc.all_engine_barrier()
        assert self.sems is not None
        popped = nc._tile_sem_poison_stack.pop()
        assert popped is self._sem_poison
        nc.clear_and_free_semaphores(list(self.sems.allocated().values()))
        nc.all_engine_barrier()

    _tile.TileContext._drain_and_barrier = _patched_drain_and_barrier
    _tile.TileContext._drain_patch_installed = True


def _split_excess_waits(nc, max_waits=_MAX_WAITS):
    """Drop same-engine self-waits (satisfied by in-order execution), then
    hoist remaining excess on_wait entries onto same-engine NoOps."""
    import concourse.mybir as mybir

    counter = 0
    for fn in nc.m.functions:
        for bb in fn.blocks:
            new = []
            dirty = False
            for inst in bb.instructions:
                si = inst.sync_info
                waits = list(si.on_wait) if si and si.on_wait else []
                if len(waits) > max_waits:
                    eng_name = str(inst.engine).split(".")[-1]
                    kept = [
                        w
                        for w in waits
                        if not (
                            w.ant_name
                            and w.ant_name.rsplit("_", 1)[0] == eng_name
                        )
                    ]
                    if len(kept) != len(waits):
                        dirty = True
                        waits = kept
                        inst.sync_info = mybir.SyncInfo(
                            on_wait=list(waits), on_update=list(si.on_update or [])
                        )
                        si = inst.sync_info
                if len(waits) > max_waits:
                    dirty = True
                    excess, keep = waits[:-max_waits], waits[-max_waits:]
                    for i in range(0, len(excess), max_waits):
                        counter += 1
                        nop = mybir.InstNoOp(
                            name=f"waitsplit-{counter}", ins=[], outs=[]
                        )
                        nop.engine = inst.engine
                        nop.sync_info = mybir.SyncInfo(
                            on_wait=excess[i : i + max_waits], on_update=[]
                        )
                        new.append(nop)
                    inst.sync_info = mybir.SyncInfo(
                        on_wait=keep, on_update=list(si.on_update or [])
                    )
                new.append(inst)
            if dirty:
                bb.instructions = new
    return counter


def _build_nc(n_a=43, n_p=14, n_ch=2, loop_n=1, t_bufs=6, d_bufs=5,
              relu_eng="scalar", lag1=2, lag2=4, merge_lag=5,
              prep_engines=("scalar", "vector", "gpsimd")):
    """Build the per-core Bass program.

    Per filter: 100 taps = 1 DVE accD init + n_ch DVE accP inits +
    n_a A-taps + n_p P-taps + rest B-taps. P-tap stage lags (in master
    tap slots): relu at +lag1, Pool update at +lag2. Chain merges of
    filter f are emitted merge_lag slots into filter f+1.
    loop_n>1 wraps the compute body in a hardware loop (timing only;
    the body is idempotent).
    """
    import concourse.bass as bass
    import concourse.mybir as mybir
    from concourse import tile
    from contextlib import ExitStack

    _install_tile_drain_patch()

    f16 = mybir.dt.float16
    f32 = mybir.dt.float32
    NK = _KH * _KW * _C * _F  # 800
    n_b = 100 - 1 - n_ch - n_a - n_p
    assert n_b >= 0, (n_a, n_p, n_ch)

    nc = bass.Bass()
    x = nc.declare_dram_parameter("x", [_BL, _H, _W, _C], f32, isOutput=False)
    k = nc.declare_dram_parameter("k", [_KH, _KW, _C, _F], f32, isOutput=False)
    y = nc.declare_dram_parameter("y", [_BL, _HO, _WO, _F], f16, isOutput=True)

    with tile.TileContext(nc) as tc:
        with (
            tc.tile_pool(name="planes", bufs=1) as pp,
            tc.tile_pool(name="kpool", bufs=1) as kp,
        ):
            k_rep = kp.tile([128, NK], f32, tag="krep")
            nc.sync.dma_start(
                out=k_rep[:],
                in_=k[:].flatten().unsqueeze(0).broadcast_to((128, NK)),
            )
            kneg = kp.tile([128, NK], f32, tag="kneg")
            nc.vector.tensor_scalar_mul(out=kneg[:], in0=k_rep[:], scalar1=-1.0)

            planes = {}
            with tc.tile_pool(name="raw", bufs=2) as rp:
                prep_i = 0
                for dy in range(_KH):
                    raw = rp.tile([_P, _J, _BL, _W * _C], f32, tag="raw")
                    src = x[:, dy : dy + _P * _J].rearrange(
                        "b (j p) w c -> p j b (w c)", j=_J, p=_P
                    )
                    for j in range(_J):
                        nc.sync.dma_start(out=raw[:, j], in_=src[:, j])
                    for c in range(_C):
                        pl = pp.tile([_P, _J, _W, _BL], f16, tag=f"plane_{dy}_{c}")
                        eng = prep_engines[prep_i % len(prep_engines)]
                        prep_i += 1
                        src_ap = raw[:, :, :, c :: _C].rearrange(
                            "p j b w -> p j w b"
                        )
                        if eng == "scalar":
                            nc.scalar.copy(out=pl[:], in_=src_ap)
                        elif eng == "vector":
                            nc.vector.tensor_copy(out=pl[:], in_=src_ap)
                        else:
                            nc.gpsimd.tensor_scalar(
                                out=pl[:], in0=src_ap, scalar1=0.0,
                                scalar2=None, op0=mybir.AluOpType.add,
                            )
                        planes[(dy, c)] = pl

            with (
                tc.tile_pool(name="accp", bufs=2) as ap_,
                tc.tile_pool(name="tp", bufs=t_bufs) as tp,
                tc.tile_pool(name="dp", bufs=d_bufs) as dp,
                tc.tile_pool(name="outp", bufs=1) as op_,
                ExitStack() as loop_ctx,
            ):
                if loop_n > 1:
                    loop_ctx.enter_context(tc.For_i(0, loop_n, 1))
                out_stage = op_.tile([_P, _J, _BL, _WO, _F], f16, tag="out")

                taps = [
                    (dy, dx, c)
                    for dy in range(_KH)
                    for dx in range(_KW)
                    for c in range(_C)
                ]

                def kidx(dy, dx, c, f):
                    return ((dy * _KW + dx) * _C + c) * _F + f

                def win(dy, dx, c):
                    return planes[(dy, c)][:, :, dx : dx + _WO, :]

                # master schedule: weave A, P, B taps at even pacing
                def weave():
                    rest = taps[1 + n_ch :]
                    assert len(rest) == n_a + n_p + n_b
                    kinds = []
                    counts = {"A": n_a, "P": n_p, "B": n_b}
                    acc = {kk: 0.0 for kk in counts}
                    done = {kk: 0 for kk in counts}
                    total = n_a + n_p + n_b
                    for _ in range(total):
                        # pick kind with largest remaining deficit
                        best, bestv = None, -1.0
                        for kk, cnt in counts.items():
                            if done[kk] >= cnt:
                                continue
                            v = cnt / total * (len(kinds) + 1) - done[kk]
                            if v > bestv:
                                best, bestv = kk, v
                        kinds.append(best)
                        done[best] += 1
                    return list(zip(kinds, rest))

                master = weave()
                g_idx = 0
                pend = []  # (due, seq, fn)
                seq = [0]

                def later(delay, fn):
                    seq[0] += 1
                    heapq.heappush(pend, (g_idx + delay, seq[0], fn))

                def flush(upto=None):
                    while pend and (upto is None or pend[0][0] <= upto):
                        heapq.heappop(pend)[2]()

                ts_sub = lambda out, dy, dx, c, f: nc.vector.tensor_scalar(
                    out=out, in0=win(dy, dx, c),
                    scalar1=k_rep[0:_P, kidx(dy, dx, c, f) : kidx(dy, dx, c, f) + 1],
                    scalar2=None, op0=mybir.AluOpType.subtract,
                )

                for f in range(_F):
                    accD = ap_.tile([_P, _J, _WO, _BL], f16, tag="accD",
                                    name=f"accD_{f}")
                    accPs = [
                        ap_.tile([_P, _J, _WO, _BL], f16, tag=f"accP{c}",
                                 name=f"accP{c}_{f}")
                        for c in range(n_ch)
                    ]
                    ts_sub(accD[:], *taps[0], f)
                    for c in range(n_ch):
                        ts_sub(accPs[c][:], *taps[1 + c], f)
                    p_i = 0
                    for kind, tap in master:
                        flush(g_idx)
                        dy, dx, c = tap
                        idx = kidx(dy, dx, c, f)
                        if kind == "A":
                            t = tp.tile([_P, _J, _WO, _BL], f16, tag="t")
                            nc.scalar.add(
                                out=t[:], in_=win(dy, dx, c),
                                add=kneg[0:_P, idx : idx + 1],
                            )
                            nc.vector.tensor_tensor(
                                out=accD[:], in0=t[:], in1=accD[:],
                                op=mybir.AluOpType.min,
                            )
                        elif kind == "B":
                            t = tp.tile([_P, _J, _WO, _BL], f16, tag="t")
                            ts_sub(t[:], dy, dx, c, f)
                            nc.vector.tensor_tensor(
                                out=accD[:], in0=t[:], in1=accD[:],
                                op=mybir.AluOpType.min,
                            )
                        else:  # P-tap: acc - relu(acc - x + k), pool chain
                            acc = accPs[p_i % n_ch]
                            p_i += 1
                            d = dp.tile([_P, _J, _WO, _BL], f16, tag="d")
                            r = dp.tile([_P, _J, _WO, _BL], f16, tag="r")
                            nc.gpsimd.tensor_tensor(
                                out=d[:], in0=acc[:], in1=win(dy, dx, c),
                                op=mybir.AluOpType.subtract,
                            )

                            def do_relu(d=d, r=r, idx=idx):
                                if relu_eng == "scalar":
                                    nc.scalar.activation(
                                        out=r[:], in_=d[:],
                                        func=mybir.ActivationFunctionType.Relu,
                                        bias=k_rep[0:_P, idx : idx + 1],
                                        scale=1.0,
                                    )
                                else:
                                    nc.vector.tensor_scalar(
                                        out=r[:], in0=d[:],
                                        scalar1=k_rep[0:_P, idx : idx + 1],
                                        scalar2=0.0,
                                        op0=mybir.AluOpType.add,
                                        op1=mybir.AluOpType.max,
                                    )

                            def do_upd(acc=acc, r=r):
                                nc.gpsimd.tensor_tensor(
                                    out=acc[:], in0=acc[:], in1=r[:],
                                    op=mybir.AluOpType.subtract,
                                )

                            later(lag1, do_relu)
                            later(lag2, do_upd)
                        g_idx += 1

                    # chain merges, delayed into next filter's stream
                    def do_merges(accD=accD, accPs=accPs, f=f):
                        for c in range(n_ch - 1):
                            nc.vector.tensor_tensor(
                                out=accD[:], in0=accPs[c][:], in1=accD[:],
                                op=mybir.AluOpType.min,
                            )
                        nc.vector.tensor_tensor(
                            out=out_stage[:, :, :, :, f].rearrange(
                                "p j b x -> p j x b"
                            ),
                            in0=accPs[n_ch - 1][:],
                            in1=accD[:],
                            op=mybir.AluOpType.min,
                        )

                    later(merge_lag, do_merges)
                flush()
                ydst = y[:].rearrange("b (j p) x f -> p j b (x f)", j=_J, p=_P)
                for j in range(_J):
                    nc.sync.dma_start(out=ydst[:, j], in_=out_stage[:, j])

    _split_excess_waits(nc)
    return nc


_cache = {}


def kernel(**inputs):
    x = np.ascontiguousarray(np.asarray(inputs["x"]), dtype=np.float32)
    k = np.ascontiguousarray(np.asarray(inputs["kernel"]), dtype=np.float32)
    assert x.shape == (_B, _H, _W, _C) and k.shape == (_KH, _KW, _C, _F)

    from concourse.bass_utils import run_bass_kernel_spmd

    if "nc" not in _cache:
        _cache["nc"] = _build_nc()
    nc = _cache["nc"]

    xs = x.reshape(_NCORES, _BL, _H, _W, _C)
    in_maps = [{"x": xs[i], "k": k} for i in range(_NCORES)]
    res = run_bass_kernel_spmd(
        nc, in_maps, core_ids=list(range(_NCORES)), **_cache.get("run_kwargs", {})
    )
    _cache["last_results"] = res
    out = np.concatenate([r["y"][None] for r in res.results], axis=0)
    return out.reshape(_B, _HO, _WO, _F).astype(np.float32)
